# revision 13
# baseline (speedup 1.0000x reference)
"""Viterbi CRF decode kernel for Trainium2 (8 NeuronCores, Bass/Tile).

Problem: feats [S=2048, B=4096, T=10] fp32, transitions [10, 10] fp32
(transitions[next, prev]; row START=8 and col STOP=9 are -1e4).
Returns (path_score [B] fp32, best_path [S, B] int32) exactly matching
the jax reference (including fp32 rounding and argmax tie-breaking).

Key exact reduction: for t >= 1 the argmax over prev can never select
prev in {8, 9} (margin ~1e4), and next tags 8/9 are never on the decoded
path, so the scan runs on an 8x8 tag block.  t = 0 is special-cased:
fv1[n] = A[n, 8] + feat[0, n] (prev=START wins with margin ~1e4).

Sharding: data-parallel over batch, 512 batches per core; the sequential
scan over seq_len stays local per core.
"""

import os
import sys

import numpy as np

sys.path.insert(0, "/opt/trn_rl_repo")

S_FULL = 2048
B_FULL = 4096
T_FULL = 10
N_CORES = 8
B_CORE = B_FULL // N_CORES  # 512
G = B_CORE // 128  # 4 batch groups per core
NT = 8  # reduced tag count (tags 0..7)
CHUNK = 32  # timesteps per phase-B block

_LAST_RESULTS = None  # BassKernelResults of the last run (for test harness)


def _build_kernel(S):
    import concourse.bacc as bacc
    import concourse.mybir as mybir
    from concourse.tile import TileContext

    fp32 = mybir.dt.float32
    i8 = mybir.dt.int8
    Alu = mybir.AluOpType
    X = mybir.AxisListType.X

    nc = bacc.Bacc(
        "TRN2",
        target_bir_lowering=False,
        debug=False,
        enable_asserts=False,
        num_devices=N_CORES,
    )

    # DRAM I/O (per-core shapes)
    featsr = nc.dram_tensor("featsr", [128, S, G * NT], fp32, kind="ExternalInput").ap()
    transb = nc.dram_tensor("transb", [128, G * NT * NT], fp32, kind="ExternalInput").ap()
    wdesc = nc.dram_tensor("wdesc", [128, G * NT * NT], fp32, kind="ExternalInput").ap()
    acol8 = nc.dram_tensor("acol8", [128, G * NT], fp32, kind="ExternalInput").ap()
    a9row = nc.dram_tensor("a9row", [128, G * NT], fp32, kind="ExternalInput").ap()
    iotadesc = nc.dram_tensor("iotadesc", [128, G * NT], i8, kind="ExternalInput").ap()
    wdesc32 = nc.dram_tensor("wdesc32", [128, G * NT], fp32, kind="ExternalInput").ap()

    path_enc = nc.dram_tensor("path_enc", [128, S, G], i8, kind="ExternalOutput").ap()
    ps_out = nc.dram_tensor("ps_out", [128, G], fp32, kind="ExternalOutput").ap()

    n_chunks = (S + CHUNK - 1) // CHUNK
    assert S % CHUNK == 0

    with TileContext(nc) as tc:
        with tc.tile_pool(name="persist", bufs=1) as pp, \
             tc.tile_pool(name="feats", bufs=2) as fpool:
            # persistent tiles
            fv = pp.tile([128, G * NT], fp32, tag="fv")
            scores = pp.tile([128, CHUNK, G * NT * NT], fp32, tag="scores")
            maxvh = pp.tile([128, CHUNK, G * NT], fp32, tag="maxvh")
            bp = pp.tile([128, S, G * NT], i8, tag="bp")
            path = pp.tile([128, S, G], i8, tag="path")
            tb = pp.tile([128, G * NT * NT], fp32, tag="tb")
            wd = pp.tile([128, G * NT * NT], fp32, tag="wd")
            ac8 = pp.tile([128, G * NT], fp32, tag="ac8")
            a9r = pp.tile([128, G * NT], fp32, tag="a9r")
            iod = pp.tile([128, G * NT], i8, tag="iod")
            wd32 = pp.tile([128, G * NT], fp32, tag="wd32")
            term = pp.tile([128, G * NT], fp32, tag="term")
            tmax = pp.tile([128, G], fp32, tag="tmax")
            eqbt = pp.tile([128, G * NT], i8, tag="eqbt")

            nc.sync.dma_start(out=tb, in_=transb)
            nc.sync.dma_start(out=wd, in_=wdesc)
            nc.sync.dma_start(out=ac8, in_=acol8)
            nc.sync.dma_start(out=a9r, in_=a9row)
            nc.sync.dma_start(out=iod, in_=iotadesc)
            nc.sync.dma_start(out=wd32, in_=wdesc32)

            feat_tiles = []
            for c in range(n_chunks):
                ft = fpool.tile([128, CHUNK, G * NT], fp32, tag="ft")
                nc.sync.dma_start(out=ft, in_=featsr[:, c * CHUNK:(c + 1) * CHUNK, :])
                feat_tiles.append(ft)

            # ---- forward scan + chunked backpointer extraction ----
            # slot 0 of chunk 0 is never written by the scan; keep it finite
            nc.vector.memset(scores[:, 0, :], 0.0)
            nc.vector.memset(maxvh[:, 0, :], 0.0)

            for c in range(n_chunks):
                ft = feat_tiles[c]
                for o in range(CHUNK):
                    t = c * CHUNK + o
                    if t == 0:
                        # fv1 = A[:, 8] + feat[0]  (prev=START wins exactly)
                        nc.vector.tensor_tensor(
                            out=fv, in0=ac8, in1=ft[:, 0, :], op=Alu.add)
                        continue
                    # scores[t] = fv[prev] + A[next, prev]
                    nc.vector.tensor_tensor(
                        out=scores[:, o, :],
                        in0=fv.rearrange("p (g q) -> p g q", g=G)
                            .unsqueeze(2).broadcast_to([128, G, NT, NT]),
                        in1=tb.rearrange("p (g n q) -> p g n q", g=G, n=NT),
                        op=Alu.add)
                    # maxv[t] = max over prev
                    nc.vector.reduce_max(
                        out=maxvh[:, o, :],
                        in_=scores[:, o, :].rearrange("p (m q) -> p m q", q=NT),
                        axis=X)
                    # fv = maxv + feat[t]
                    nc.vector.tensor_tensor(
                        out=fv, in0=maxvh[:, o, :], in1=ft[:, o, :], op=Alu.add)

                # ---- phase B for this chunk: bp = argmax (first occurrence) ----
                # eq = (scores == maxv), in place
                nc.vector.tensor_tensor(
                    out=scores.rearrange("p c (m q) -> p c m q", q=NT),
                    in0=scores.rearrange("p c (m q) -> p c m q", q=NT),
                    in1=maxvh.unsqueeze(3)
                        .broadcast_to([128, CHUNK, G * NT, NT]),
                    op=Alu.is_equal)
                # cand = eq * (8 - prev), in place
                nc.vector.tensor_tensor(
                    out=scores,
                    in0=scores,
                    in1=wd.unsqueeze(1).broadcast_to(
                        [128, CHUNK, G * NT * NT]),
                    op=Alu.mult)
                # bp_enc = max over prev of cand  (= 8 - first-occurrence argmax)
                nc.vector.reduce_max(
                    out=bp[:, c * CHUNK:(c + 1) * CHUNK, :],
                    in_=scores.rearrange("p c (m q) -> p (c m) q", q=NT),
                    axis=X)

            # ---- terminal / path_score / best_last ----
            nc.vector.tensor_tensor(out=term, in0=fv, in1=a9r, op=Alu.add)
            nc.vector.reduce_max(
                out=tmax, in_=term.rearrange("p (g n) -> p g n", g=G), axis=X)
            nc.sync.dma_start(out=ps_out, in_=tmax)
            # best_last_enc: eq * (8 - n), reduce max
            nc.vector.tensor_tensor(
                out=term.rearrange("p (g n) -> p g n", g=G),
                in0=term.rearrange("p (g n) -> p g n", g=G),
                in1=tmax.unsqueeze(2).broadcast_to([128, G, NT]),
                op=Alu.is_equal)
            nc.vector.tensor_tensor(
                out=term, in0=term, in1=wd32, op=Alu.mult)
            # write enc(best_last) into path[S-1]
            nc.vector.reduce_max(
                out=path[:, S - 1, :],
                in_=term.rearrange("p (g n) -> p g n", g=G), axis=X)

            # ---- backtrack ----
            for t in range(S - 1, 0, -1):
                # eqbt[j] = (iotadesc[j] == tag_enc)  <=> j == tag
                nc.vector.tensor_tensor(
                    out=eqbt.rearrange("p (g n) -> p g n", g=G),
                    in0=iod.rearrange("p (g n) -> p g n", g=G),
                    in1=path[:, t, :].unsqueeze(2).broadcast_to([128, G, NT]),
                    op=Alu.is_equal)
                # prod = eqbt * bp[t], in place
                nc.vector.tensor_tensor(
                    out=eqbt, in0=eqbt, in1=bp[:, t, :], op=Alu.mult)
                # tag_enc' = max over j (single nonzero)
                nc.vector.reduce_max(
                    out=path[:, t - 1, :],
                    in_=eqbt.rearrange("p (g n) -> p g n", g=G), axis=X)

            nc.sync.dma_start(out=path_enc, in_=path)

    nc.compile()
    return nc


def _host_prep(feats, transitions, S):
    A = np.asarray(transitions, dtype=np.float32)
    feats = np.asarray(feats, dtype=np.float32)

    # per-core feats: [128, S, G, 8] p-major, contiguous
    # b = core*512 + g*128 + p
    f8 = feats[:, :, :NT]                                  # [S, B, 8]
    f8 = f8.reshape(S, N_CORES, G, 128, NT)                # [S, c, g, p, n]
    f8 = np.ascontiguousarray(f8.transpose(1, 3, 0, 2, 4)) # [c, p, S, g, n]
    featsr = f8.reshape(N_CORES, 128, S, G * NT)

    A8 = A[:NT, :NT]                                       # [next, prev]
    transb = np.tile(A8.reshape(-1), G)[None, :].repeat(128, 0).astype(np.float32)
    wdesc = np.tile(np.tile(8.0 - np.arange(NT, dtype=np.float32), NT), G)
    wdesc = wdesc[None, :].repeat(128, 0).astype(np.float32)
    acol8 = np.tile(A[:NT, 8], G)[None, :].repeat(128, 0).astype(np.float32)
    a9row = np.tile(A[9, :NT], G)[None, :].repeat(128, 0).astype(np.float32)
    iotadesc = np.tile(8 - np.arange(NT, dtype=np.int64), G).astype(np.int8)
    iotadesc = iotadesc[None, :].repeat(128, 0)
    wdesc32 = np.tile(8.0 - np.arange(NT, dtype=np.float32), G)
    wdesc32 = wdesc32[None, :].repeat(128, 0).astype(np.float32)

    consts = dict(transb=transb, wdesc=wdesc, acol8=acol8, a9row=a9row,
                  iotadesc=iotadesc, wdesc32=wdesc32)
    return featsr, consts


def kernel(feats, transitions):
    global _LAST_RESULTS
    from concourse.bass_utils import run_bass_kernel_spmd

    feats = np.asarray(feats)
    S = feats.shape[0]
    nc = _build_kernel(S)
    featsr, consts = _host_prep(feats, transitions, S)

    in_maps = []
    for c in range(N_CORES):
        m = {"featsr": featsr[c]}
        m.update(consts)
        in_maps.append(m)

    trace = bool(int(os.environ.get("CRF_TRACE", "0")))
    res = run_bass_kernel_spmd(nc, in_maps, core_ids=list(range(N_CORES)),
                               trace=trace)
    _LAST_RESULTS = res

    path_score = np.empty(B_FULL, dtype=np.float32)
    best_path = np.empty((S, B_FULL), dtype=np.int32)
    for c in range(N_CORES):
        enc = res.results[c]["path_enc"]      # [128, S, G] int8
        ps = res.results[c]["ps_out"]         # [128, G] fp32
        dec = (8 - enc.astype(np.int32)).transpose(1, 2, 0)  # [S, G, 128]
        best_path[:, c * B_CORE:(c + 1) * B_CORE] = dec.reshape(S, B_CORE)
        path_score[c * B_CORE:(c + 1) * B_CORE] = ps.T.reshape(B_CORE)
    return path_score, best_path


if __name__ == "__main__":
    # smoke test with small S against a numpy reference
    S = int(sys.argv[1]) if len(sys.argv) > 1 else 64
    rng = np.random.default_rng(0)
    feats = rng.standard_normal((S, B_FULL, T_FULL), dtype=np.float32)
    A = rng.standard_normal((T_FULL, T_FULL), dtype=np.float32)
    A[8, :] = -10000.0
    A[:, 9] = -10000.0
    ps, bp = kernel(feats, A)
    print("kernel done", ps[:4], bp[:4, :4])
